# revision 4
# baseline (speedup 1.0000x reference)
"""Bayesian NN Monte-Carlo sampling kernel for 8 TRN2 NeuronCores.

Shards the n_samples axis (S=100 -> 13 per core, 4 padded) across 8 cores.
All math is general (std computed on device from the logvar tensors); host
prep is layout/dtype-only (bf16 cast + reshape/transpose).

Layout: features interleaved mod 4, contraction rows grouped p-major on the
host, and samples concatenated along the free dim, so eps weights stream as
multi-sample grouped DMAs with 7-28KB contiguous per-partition lines
(amortizes per-descriptor HBM latency; ~2x the effective GB/s of 1KB lines).
Each layer's relu output lands exactly in the next layer's contraction
layout (partition p holds features 4p..4p+3) -> no transposes anywhere.

Engine split per sample (all matmuls bf16):
  DVE: one in-place group mul per eps stream (w = eps*std), plus the t0-1
       half of the layer-1 mean fold; GPSIMD adds the t2-3 half.
  PE:  psum[128,256] per layer; layer-0 psum is initialized with the
       precomputed y0T = x@wm0 via an identity matmul, so the DVE never
       waits on PSUM and relus read psum directly.
  ACT: per-chunk biased relus; single output copy at the end.
"""

import os
import sys

import numpy as np

if "/opt/trn_rl_repo" not in sys.path:
    sys.path.insert(0, "/opt/trn_rl_repo")

import concourse.bass as bass
from concourse import bacc, mybir, tile
from concourse.bass_utils import run_bass_kernel_spmd

S, B = 100, 64
D0, D1, D2, DO = 784, 512, 512, 10
NCORES = 8
SP = 13           # samples per core; 8*13 = 104, last 4 are wrap padding
P0, T0 = 112, 7   # layer-0 contraction: k = 7*p + t (p-major)
P1, T1 = 128, 4   # layer-1/2 contraction: k = 4*p + t (p-major)
C1 = 4            # feature chunks (features 4*q + c on chunk c, partition q)
W0C, W1C = T0 * D1, T1 * D2   # per-sample eps columns: 3584, 2048
GROUPS = [(0, 1), (1, 3), (3, 6), (6, 9), (9, 13)]
GMAX = 4

F32 = mybir.dt.float32
F32R = mybir.dt.float32r
BF16 = mybir.dt.bfloat16

_CACHE = {}


def _build(mode="bf16"):
    io_dt = BF16
    ts = bass.ts
    AF = mybir.ActivationFunctionType

    nc = bacc.Bacc("TRN2", target_bir_lowering=False, debug=False,
                   num_devices=NCORES)

    def inp(name, shape, dt=io_dt):
        return nc.dram_tensor(name, shape, dt, kind="ExternalInput").ap()

    # p-major / mod-4-interleaved host layouts (see _prep_in_maps)
    xT = inp("xT", [P0, T0 * B])
    wm0 = inp("wm0", [P0, W0C])
    wv0 = inp("wv0", [P0, W0C])
    wm1 = inp("wm1", [P1, W1C])
    wv1 = inp("wv1", [P1, W1C])
    wmlT = inp("wmlT", [P1, T1 * DO])
    wvlT = inp("wvlT", [P1, T1 * DO])
    welT = inp("welT", [P1, SP * T1 * DO])
    we0A = inp("we0A", [P0, SP * W0C])   # samples concatenated per partition
    we1A = inp("we1A", [P1, SP * W1C])
    eye128 = inp("eye128", [P1, P1])

    bv0T = inp("bv0T", [P1, C1], F32)
    bm0T = inp("bm0T", [P1, C1], F32)
    be0T = inp("be0T", [P1, C1 * SP], F32)
    bv1T = inp("bv1T", [P1, C1], F32)
    bm1T = inp("bm1T", [P1, C1], F32)
    be1T = inp("be1T", [P1, C1 * SP], F32)
    bvl = inp("bvl", [1, DO])
    bml = inp("bml", [1, DO])
    bel = inp("bel", [SP, DO])
    ones13 = inp("ones13", [1, SP])
    ind = inp("ind", [SP, SP * B])
    out = nc.dram_tensor("out", [B, SP * DO], F32, kind="ExternalOutput").ap()

    with tile.TileContext(nc) as tc:
        with tc.tile_pool(name="const", bufs=1) as const, \
             tc.tile_pool(name="w0g", bufs=2) as w0g, \
             tc.tile_pool(name="w1g", bufs=2) as w1g, \
             tc.tile_pool(name="wls", bufs=2) as wls, \
             tc.tile_pool(name="acts", bufs=2) as acts, \
             tc.tile_pool(name="bias", bufs=1) as bias, \
             tc.tile_pool(name="ps0", bufs=2, space="PSUM") as ps0, \
             tc.tile_pool(name="ps1", bufs=2, space="PSUM") as ps1, \
             tc.tile_pool(name="ps_b", bufs=1, space="PSUM") as ps_b, \
             tc.tile_pool(name="ps_o", bufs=1, space="PSUM") as ps_o:

            # ---------------- one-time setup ----------------
            tmp0 = const.tile([P0, W0C], io_dt, tag="tmp0")
            nc.sync.dma_start(tmp0[:], wv0[:, :])
            t_std0 = const.tile([P0, W0C], io_dt)
            nc.scalar.activation(t_std0[:], tmp0[:], AF.Exp, scale=0.5)

            tmp1 = const.tile([P1, W1C], io_dt, tag="tmp1")
            nc.scalar.dma_start(tmp1[:], wv1[:, :])
            t_std1 = const.tile([P1, W1C], io_dt)
            nc.scalar.activation(t_std1[:], tmp1[:], AF.Exp, scale=0.5)

            t_xT = const.tile([P0, T0 * B], io_dt)
            nc.scalar.dma_start(t_xT[:], xT[:, :])
            t_eye = const.tile([P1, P1], io_dt)
            nc.scalar.dma_start(t_eye[:], eye128[:, :])

            tmpl = wls.tile([P1, T1 * DO], io_dt, tag="t_wls")
            nc.scalar.dma_start(tmpl[:], wvlT[:, :])
            t_stdl = const.tile([P1, T1 * DO], io_dt)
            nc.scalar.activation(t_stdl[:], tmpl[:], AF.Exp, scale=0.5)
            t_wml = const.tile([P1, T1 * DO], io_dt)
            nc.scalar.dma_start(t_wml[:], wmlT[:, :])
            t_wel = const.tile([P1, SP * T1 * DO], io_dt)
            nc.scalar.dma_start(t_wel[:], welT[:, :])

            # chunk-layout hidden biases: bT[q, c*SP+s] = b_s[4q+c]
            def make_bias_T(bvT, bmT, beT, name):
                vt = bias.tile([P1, C1], F32, tag="vT")
                nc.scalar.dma_start(vt[:], bvT[:, :])
                st = bias.tile([P1, C1], F32, tag="sT")
                nc.scalar.activation(st[:], vt[:], AF.Exp, scale=0.5)
                mt = bias.tile([P1, C1], F32, tag="mT")
                nc.scalar.dma_start(mt[:], bmT[:, :])
                et = bias.tile([P1, C1 * SP], F32, tag="eT")
                nc.scalar.dma_start(et[:], beT[:, :])
                bt = const.tile([P1, C1 * SP], F32, tag=name)
                for c in range(C1):
                    nc.vector.tensor_scalar_mul(
                        bt[:, ts(c, SP)], et[:, ts(c, SP)], st[:, c:c + 1])
                    nc.vector.tensor_scalar_add(
                        bt[:, ts(c, SP)], bt[:, ts(c, SP)], mt[:, c:c + 1])
                return bt

            t_bT0 = make_bias_T(bv0T, bm0T, be0T, "bT0")
            t_bT1 = make_bias_T(bv1T, bm1T, be1T, "bT1")

            # last-layer bias rows [SP, DO]: ones-matmul broadcast
            t_ones13 = const.tile([1, SP], io_dt)
            nc.scalar.dma_start(t_ones13[:], ones13[:, :])

            def bcast(row, D, tag):
                pb = ps_b.tile([SP, D], F32, tag="bb")
                nc.tensor.matmul(pb[:], t_ones13[:], row[:],
                                 start=True, stop=True)
                sbuf = bias.tile([SP, D], io_dt, tag=tag)
                nc.scalar.copy(sbuf[:], pb[:])
                return sbuf

            r = bias.tile([1, DO], io_dt, tag="brow")
            nc.scalar.dma_start(r[:], bvl[:, :])
            sb = bias.tile([1, DO], io_dt, tag="brow2")
            nc.scalar.activation(sb[:], r[:], AF.Exp, scale=0.5)
            sbb = bcast(sb, DO, "bb1")
            mr = bias.tile([1, DO], io_dt, tag="brow3")
            nc.scalar.dma_start(mr[:], bml[:, :])
            mb = bcast(mr, DO, "bb2")
            eb = bias.tile([SP, DO], io_dt, tag="bb3")
            nc.scalar.dma_start(eb[:], bel[:, :])
            ba = bias.tile([SP, DO], io_dt, tag="bb4")
            nc.vector.tensor_mul(ba[:], eb[:], sbb[:])
            t_bl = bias.tile([SP, DO], io_dt, tag="ball")
            nc.vector.tensor_add(t_bl[:], ba[:], mb[:])

            t_ind = const.tile([SP, SP * B], io_dt)
            nc.scalar.dma_start(t_ind[:], ind[:, :])

            t_wm0 = const.tile([P0, W0C], io_dt)
            t_wm1 = const.tile([P1, W1C], io_dt)

            t_out = const.tile([B, SP * DO], F32)

            def mm(psum, lhsT, rhs, start, stop, skip=False):
                nc.tensor.matmul(psum, lhsT, rhs, start=start, stop=stop,
                                 skip_group_check=skip)

            # y0T[q, c*64+b] = (x @ wm0)[4q+c, b], precomputed once (bf16)
            def make_y0T():
                py0 = ps0.tile([P1, C1 * B], F32, tag="p0")
                for c in range(C1):
                    for t in range(T0):
                        mm(py0[:, ts(c, B)],
                           t_wm0[:, t * D1 + c * P1: t * D1 + (c + 1) * P1],
                           t_xT[:, ts(t, B)],
                           start=(t == 0), stop=(t == T0 - 1))
                y0 = const.tile([P1, C1 * B], io_dt)
                nc.scalar.copy(y0[:], py0[:])
                return y0

            # ---------------- grouped weight prep ----------------
            def weight_prep(gi, first=False):
                s0, s1 = GROUPS[gi]
                g = s1 - s0
                t_e0 = w0g.tile([P0, GMAX * W0C], io_dt, tag="t_e0")
                nc.sync.dma_start(t_e0[:, : g * W0C],
                                  we0A[:, s0 * W0C: s1 * W0C])
                if first:
                    nc.sync.dma_start(t_wm0[:], wm0[:, :])
                t_e1 = w1g.tile([P1, GMAX * W1C], io_dt, tag="t_e1")
                nc.scalar.dma_start(t_e1[:, : g * W1C],
                                    we1A[:, s0 * W1C: s1 * W1C])
                if first:
                    nc.scalar.dma_start(t_wm1[:], wm1[:, :])

                # in-place group dequant: w = eps * std
                e0v = t_e0[:, : g * W0C].rearrange("p (s c) -> p s c", c=W0C)
                nc.vector.tensor_mul(
                    e0v, e0v,
                    t_std0[:].unsqueeze(1).broadcast_to([P0, g, W0C]))
                e1v = t_e1[:, : g * W1C].rearrange("p (s c) -> p s c", c=W1C)
                nc.vector.tensor_mul(
                    e1v, e1v,
                    t_std1[:].unsqueeze(1).broadcast_to([P1, g, W1C]))
                # layer-1 mean fold, split DVE (t0-1) / GPSIMD (t2-3)
                H = W1C // 2
                lo = t_e1[:, : g * W1C].rearrange(
                    "p (s c) -> p s c", c=W1C)[:, :, :H]
                nc.vector.tensor_add(
                    lo, lo, t_wm1[:, :H].unsqueeze(1).broadcast_to([P1, g, H]))
                hi = t_e1[:, : g * W1C].rearrange(
                    "p (s c) -> p s c", c=W1C)[:, :, H:]
                nc.gpsimd.tensor_add(
                    hi, hi, t_wm1[:, H:].unsqueeze(1).broadcast_to([P1, g, H]))

                # last layer: wlf_s = wel_s * stdl + wml (small)
                t_wl = wls.tile([P1, GMAX * T1 * DO], io_dt, tag="t_wlf")
                wlv = t_wl[:, : g * T1 * DO].rearrange(
                    "p (s c) -> p s c", c=T1 * DO)
                nc.vector.tensor_mul(
                    wlv, t_wel[:, s0 * T1 * DO: s1 * T1 * DO].rearrange(
                        "p (s c) -> p s c", c=T1 * DO),
                    t_stdl[:].unsqueeze(1).broadcast_to([P1, g, T1 * DO]))
                nc.vector.tensor_add(
                    wlv, wlv,
                    t_wml[:].unsqueeze(1).broadcast_to([P1, g, T1 * DO]))
                return t_e0, t_e1, t_wl

            def compute(s, gi, t_e0, t_e1, t_wl, t_y0T, po):
                s0 = GROUPS[gi][0]
                j = s - s0
                w0 = t_e0[:, j * W0C: (j + 1) * W0C]
                w1 = t_e1[:, j * W1C: (j + 1) * W1C]
                wlf = t_wl[:, j * T1 * DO: (j + 1) * T1 * DO]

                # layer 0: psum initialized with y0T via identity matmul
                p0 = ps0.tile([P1, C1 * B], F32, tag="p0")
                a1T = acts.tile([P1, C1 * B], io_dt, tag="a1T")
                for c in range(C1):
                    mm(p0[:, ts(c, B)], t_eye[:], t_y0T[:, ts(c, B)],
                       start=True, stop=False)
                    for t in range(T0):
                        mm(p0[:, ts(c, B)],
                           w0[:, t * D1 + c * P1: t * D1 + (c + 1) * P1],
                           t_xT[:, ts(t, B)],
                           start=False, stop=(t == T0 - 1))
                    nc.scalar.activation(
                        a1T[:, ts(c, B)], p0[:, ts(c, B)], AF.Relu,
                        bias=t_bT0[:, c * SP + s: c * SP + s + 1])

                # layer 1 (mean already folded into w1)
                p1 = ps1.tile([P1, C1 * B], F32, tag="p1")
                a2T = acts.tile([P1, C1 * B], io_dt, tag="a2T")
                for c in range(C1):
                    for t in range(T1):
                        mm(p1[:, ts(c, B)],
                           w1[:, t * D2 + c * P1: t * D2 + (c + 1) * P1],
                           a1T[:, ts(t, B)],
                           start=(t == 0), stop=(t == T1 - 1))
                    nc.scalar.activation(
                        a2T[:, ts(c, B)], p1[:, ts(c, B)], AF.Relu,
                        bias=t_bT1[:, c * SP + s: c * SP + s + 1])

                # output layer: all samples share one [64, SP*DO] psum bank
                for t in range(T1):
                    mm(po[:, ts(s, DO)], a2T[:, ts(t, B)],
                       wlf[:, ts(t, DO)], start=(t == 0), stop=False)
                mm(po[:, ts(s, DO)], t_ind[:, ts(s, B)], t_bl[:],
                   start=False, stop=True)

            po = ps_o.tile([B, SP * DO], F32, tag="out")
            NG = len(GROUPS)
            preps = [weight_prep(0, first=True)]
            t_y0T = make_y0T()
            preps.append(weight_prep(1))
            for gi in range(NG):
                for s in range(*GROUPS[gi]):
                    compute(s, gi, *preps[gi], t_y0T, po)
                if gi + 2 < NG:
                    preps.append(weight_prep(gi + 2))

            nc.scalar.copy(t_out[:], po[:])
            nc.sync.dma_start(out[:, :], t_out[:])

    nc.compile()
    return nc


def _get_nc(mode="bf16"):
    if "nc" not in _CACHE:
        _CACHE["nc"] = _build()
    return _CACHE["nc"]


def _prep_in_maps(inputs, mode="bf16"):
    import ml_dtypes
    np_dt = ml_dtypes.bfloat16

    def cvt(a):
        return np.ascontiguousarray(a).astype(np_dt, copy=False)

    x = np.asarray(inputs["inputs"], np.float32)
    we0 = np.asarray(inputs["we0"], np.float32)
    we1 = np.asarray(inputs["we1"], np.float32)
    wel = np.asarray(inputs["wel"], np.float32)
    be0 = np.asarray(inputs["be0"], np.float32).reshape(S, D1)
    be1 = np.asarray(inputs["be1"], np.float32).reshape(S, D2)
    bel = np.asarray(inputs["bel"], np.float32).reshape(S, DO)

    # p-major rows + mod-4 interleaved feature columns:
    #   out[p, (t, c, q)] = M[T*p + t, 4*q + c]
    def pm0(M):  # [784, 512] -> [112, 7*512]
        return M.reshape(P0, T0, P1, C1).transpose(0, 1, 3, 2) \
                .reshape(P0, W0C)

    def pm1(M):  # [512, 512] -> [128, 4*512]
        return M.reshape(P1, T1, P1, C1).transpose(0, 1, 3, 2) \
                .reshape(P1, W1C)

    def pml(M):  # [512, 10] -> [128, 4*10] (row permutation only)
        return M.reshape(P1, T1 * DO)

    xTpm = x.T.reshape(P0, T0, B).reshape(P0, T0 * B)

    def bias_T(b):  # [SP, D] -> [128, C1*SP] with [q, c*SP+s] = b[s, 4q+c]
        return np.ascontiguousarray(
            b.reshape(SP, P1, C1).transpose(1, 2, 0).reshape(P1, C1 * SP))

    def bias_cq(v):  # [D] -> [128, C1] with [q, c] = v[4q+c]
        return np.ascontiguousarray(np.asarray(v, np.float32)
                                    .reshape(P1, C1))

    shared = {
        "xT": cvt(xTpm),
        "wm0": cvt(pm0(np.asarray(inputs["wm0"], np.float32))),
        "wv0": cvt(pm0(np.asarray(inputs["wv0"], np.float32))),
        "wm1": cvt(pm1(np.asarray(inputs["wm1"], np.float32))),
        "wv1": cvt(pm1(np.asarray(inputs["wv1"], np.float32))),
        "wmlT": cvt(pml(np.asarray(inputs["wml"], np.float32))),
        "wvlT": cvt(pml(np.asarray(inputs["wvl"], np.float32))),
        "eye128": cvt(np.eye(P1, dtype=np.float32)),
        "bv0T": bias_cq(inputs["bv0"]),
        "bm0T": bias_cq(inputs["bm0"]),
        "bv1T": bias_cq(inputs["bv1"]),
        "bm1T": bias_cq(inputs["bm1"]),
        "bvl": cvt(np.asarray(inputs["bvl"], np.float32).reshape(1, DO)),
        "bml": cvt(np.asarray(inputs["bml"], np.float32).reshape(1, DO)),
        "ones13": cvt(np.ones((1, SP), np.float32)),
        "ind": cvt(np.repeat(np.eye(SP, dtype=np.float32), B, axis=1)),
    }

    def shard(a, k):
        lo = k * SP
        hi = lo + SP
        if hi <= S:
            return a[lo:hi]
        return np.concatenate([a[lo:S], a[: hi - S]], axis=0)

    in_maps = []
    for k in range(NCORES):
        welk = shard(wel, k)  # [SP, 512, 10]
        in_maps.append(dict(
            shared,
            # [p, s*W0C + col] = pm0(we0_s)[p, col]  (samples along free dim)
            we0A=cvt(np.stack([pm0(m) for m in shard(we0, k)], axis=1)
                     .reshape(P0, SP * W0C)),
            we1A=cvt(np.stack([pm1(m) for m in shard(we1, k)], axis=1)
                     .reshape(P1, SP * W1C)),
            welT=cvt(np.stack([pml(m) for m in welk], axis=1)
                     .reshape(P1, SP * T1 * DO)),
            be0T=bias_T(shard(be0, k)),
            be1T=bias_T(shard(be1, k)),
            bel=cvt(shard(bel, k)),
        ))
    return in_maps


def _run(inputs, mode="bf16", trace=False):
    nc = _get_nc(mode)
    in_maps = _prep_in_maps(inputs, mode)
    res = run_bass_kernel_spmd(nc, in_maps, core_ids=list(range(NCORES)),
                               trace=trace)
    outs = []
    for k in range(NCORES):
        o = np.asarray(res.results[k]["out"], np.float32)  # [64, 130]
        outs.append(o.reshape(B, SP, DO).transpose(1, 0, 2))
    full = np.concatenate(outs, axis=0)[:S]  # [100, 64, 10]
    return full, res


def kernel(**inputs):
    out, _ = _run(inputs)
    return out


# revision 7
# speedup vs baseline: 1.1589x; 1.1589x over previous
"""Bayesian NN Monte-Carlo sampling kernel for 8 TRN2 NeuronCores.

Shards the n_samples axis (S=100 -> 13 per core, 4 padded) across 8 cores.
All math is general (std computed on device from the logvar tensors); host
prep is layout/dtype-only (bf16 cast + reshape/transpose/zero-pad).

Layout: features interleaved mod 4, contraction rows grouped p-major, and
the eps streams stored TRANSPOSED on the host so the grouped loads ride the
DMA-transpose XBAR path (higher effective GB/s than the plain-descriptor
path, which is capped ~17 GB/s per SDMA engine write-side). Each layer's
relu output lands exactly in the next layer's contraction layout (partition
p holds features 4p..4p+3) -> no transposes in the compute path.

Engine split (all matmuls bf16):
  DVE: in-place per-sample dequant muls (2x perf mode), t0-1 half of the
       layer-1 mean fold; GPSIMD adds the t2-3 half.
  PE:  psum[128,256] per layer; layer-0 psum initialized with precomputed
       y0T = x@wm0 via an identity matmul (DVE never waits on PSUM).
  ACT: per-chunk biased relus straight from psum; one output copy at end.
"""

import os
import sys

import numpy as np

if "/opt/trn_rl_repo" not in sys.path:
    sys.path.insert(0, "/opt/trn_rl_repo")

import concourse.bass as bass
from concourse import bacc, mybir, tile
from concourse.bass_utils import run_bass_kernel_spmd

S, B = 100, 64
D0, D1, D2, DO = 784, 512, 512, 10
NCORES = 8
SP = 13           # samples per core; 8*13 = 104, last 4 are wrap padding
P0, T0 = 112, 7   # layer-0 contraction: k = 7*p + t (p-major)
P1, T1 = 128, 4   # layer-1/2 contraction: k = 4*p + t (p-major)
C1 = 4            # feature chunks (features 4*q + c on chunk c, partition q)
W0C, W1C = T0 * D1, T1 * D2   # per-sample eps columns: 3584, 2048
GROUPS = [(0, 1), (1, 3), (3, 6), (6, 9), (9, 13)]
GMAX = 4
H1 = W1C // 2

F32 = mybir.dt.float32
BF16 = mybir.dt.bfloat16

_CACHE = {}


def _build(mode="bf16"):
    io_dt = BF16
    ts = bass.ts
    AF = mybir.ActivationFunctionType

    nc = bacc.Bacc("TRN2", target_bir_lowering=False, debug=False,
                   num_devices=NCORES)

    def inp(name, shape, dt=io_dt):
        return nc.dram_tensor(name, shape, dt, kind="ExternalInput").ap()

    # p-major / mod-4-interleaved host layouts (see _prep_in_maps)
    xT = inp("xT", [P0, T0 * B])
    wm0 = inp("wm0", [P0, W0C])
    wv0 = inp("wv0", [P0, W0C])
    wm1 = inp("wm1", [P1, W1C])
    wv1 = inp("wv1", [P1, W1C])
    wmlT = inp("wmlT", [P1, T1 * DO])
    wvlT = inp("wvlT", [P1, T1 * DO])
    welT = inp("welT", [P1, SP * T1 * DO])
    # eps streams, host-transposed for the DMA-transpose XBAR path
    we0AT = inp("we0AT", [SP * W0C, P1])   # cols 112-127 zero padding
    we1AT = inp("we1AT", [SP * W1C, P1])
    eye128 = inp("eye128", [P1, P1])

    b01 = inp("b01", [P1, 2 * (2 * C1 + C1 * SP)], F32)  # packed hidden biases
    bvl = inp("bvl", [1, DO])
    bml = inp("bml", [1, DO])
    bel = inp("bel", [SP, DO])
    ones13 = inp("ones13", [1, SP])
    ind = inp("ind", [SP, SP * B])
    out = nc.dram_tensor("out", [B, SP * DO], F32, kind="ExternalOutput").ap()

    with tile.TileContext(nc) as tc:
        with tc.tile_pool(name="const", bufs=1) as const, \
             tc.tile_pool(name="w0g", bufs=2) as w0g, \
             tc.tile_pool(name="w1g", bufs=2) as w1g, \
             tc.tile_pool(name="wls", bufs=2) as wls, \
             tc.tile_pool(name="acts", bufs=2) as acts, \
             tc.tile_pool(name="bias", bufs=1) as bias, \
             tc.tile_pool(name="ps0", bufs=2, space="PSUM") as ps0, \
             tc.tile_pool(name="ps1", bufs=2, space="PSUM") as ps1, \
             tc.tile_pool(name="ps_b", bufs=1, space="PSUM") as ps_b, \
             tc.tile_pool(name="ps_o", bufs=1, space="PSUM") as ps_o:

            # ---------------- one-time setup ----------------
            # sync ring: wv tensors first (they gate the sample-0 muls)
            tmp0 = const.tile([P0, W0C], io_dt, tag="tmp0")
            nc.sync.dma_start(tmp0[:], wv0[:, :])
            t_std0 = const.tile([P0, W0C], io_dt)
            nc.scalar.activation(t_std0[:], tmp0[:], AF.Exp, scale=0.5)

            tmp1 = const.tile([P1, W1C], io_dt, tag="tmp1")
            nc.sync.dma_start(tmp1[:], wv1[:, :])
            t_std1 = const.tile([P1, W1C], io_dt)
            nc.scalar.activation(t_std1[:], tmp1[:], AF.Exp, scale=0.5)

            # scalar ring: small setup tensors
            t_xT = const.tile([P0, T0 * B], io_dt)
            nc.scalar.dma_start(t_xT[:], xT[:, :])
            t_eye = const.tile([P1, P1], io_dt)
            nc.scalar.dma_start(t_eye[:], eye128[:, :])

            tmpl = wls.tile([P1, T1 * DO], io_dt, tag="t_wls")
            nc.scalar.dma_start(tmpl[:], wvlT[:, :])
            t_stdl = const.tile([P1, T1 * DO], io_dt)
            nc.scalar.activation(t_stdl[:], tmpl[:], AF.Exp, scale=0.5)
            t_wml = const.tile([P1, T1 * DO], io_dt)
            nc.scalar.dma_start(t_wml[:], wmlT[:, :])
            t_wel = const.tile([P1, SP * T1 * DO], io_dt)
            nc.scalar.dma_start(t_wel[:], welT[:, :])

            # packed hidden biases: [bv0|bm0|be0|bv1|bm1|be1] along free dim
            CB = 2 * C1 + C1 * SP
            t_b01 = bias.tile([P1, 2 * CB], F32, tag="b01")
            nc.scalar.dma_start(t_b01[:], b01[:, :])

            def make_bias_T(off, name):
                vt = t_b01[:, off: off + C1]
                mt = t_b01[:, off + C1: off + 2 * C1]
                et = t_b01[:, off + 2 * C1: off + CB]
                st = bias.tile([P1, C1], F32, tag=name + "s")
                nc.scalar.activation(st[:], vt, AF.Exp, scale=0.5)
                bt = const.tile([P1, C1 * SP], F32, tag=name)
                for c in range(C1):
                    nc.vector.tensor_scalar_mul(
                        bt[:, ts(c, SP)], et[:, ts(c, SP)], st[:, c:c + 1])
                    nc.vector.tensor_scalar_add(
                        bt[:, ts(c, SP)], bt[:, ts(c, SP)], mt[:, c:c + 1])
                return bt

            t_bT0 = make_bias_T(0, "bT0")
            t_bT1 = make_bias_T(CB, "bT1")

            # last-layer bias rows [SP, DO]: ones-matmul broadcast
            t_ones13 = const.tile([1, SP], io_dt)
            nc.scalar.dma_start(t_ones13[:], ones13[:, :])

            def bcast(row, D, tag):
                pb = ps_b.tile([SP, D], F32, tag="bb")
                nc.tensor.matmul(pb[:], t_ones13[:], row[:],
                                 start=True, stop=True)
                sbuf = bias.tile([SP, D], io_dt, tag=tag)
                nc.scalar.copy(sbuf[:], pb[:])
                return sbuf

            r = bias.tile([1, DO], io_dt, tag="brow")
            nc.scalar.dma_start(r[:], bvl[:, :])
            sb = bias.tile([1, DO], io_dt, tag="brow2")
            nc.scalar.activation(sb[:], r[:], AF.Exp, scale=0.5)
            sbb = bcast(sb, DO, "bb1")
            mr = bias.tile([1, DO], io_dt, tag="brow3")
            nc.scalar.dma_start(mr[:], bml[:, :])
            mb = bcast(mr, DO, "bb2")
            eb = bias.tile([SP, DO], io_dt, tag="bb3")
            nc.scalar.dma_start(eb[:], bel[:, :])
            ba = bias.tile([SP, DO], io_dt, tag="bb4")
            nc.vector.tensor_mul(ba[:], eb[:], sbb[:])
            t_bl = bias.tile([SP, DO], io_dt, tag="ball")
            nc.vector.tensor_add(t_bl[:], ba[:], mb[:])

            t_ind = const.tile([SP, SP * B], io_dt)
            nc.scalar.dma_start(t_ind[:], ind[:, :])

            t_wm0 = const.tile([P0, W0C], io_dt)
            t_wm1 = const.tile([P1, W1C], io_dt)

            t_out = const.tile([B, SP * DO], F32)

            def mm(psum, lhsT, rhs, start, stop, skip=False):
                nc.tensor.matmul(psum, lhsT, rhs, start=start, stop=stop,
                                 skip_group_check=skip)

            # y0T[q, c*64+b] = (x @ wm0)[4q+c, b], precomputed once (bf16)
            def make_y0T():
                py0 = ps0.tile([P1, C1 * B], F32, tag="p0")
                for c in range(C1):
                    for t in range(T0):
                        mm(py0[:, ts(c, B)],
                           t_wm0[:, t * D1 + c * P1: t * D1 + (c + 1) * P1],
                           t_xT[:, ts(t, B)],
                           start=(t == 0), stop=(t == T0 - 1))
                y0 = const.tile([P1, C1 * B], io_dt)
                nc.scalar.copy(y0[:], py0[:])
                return y0

            # ---------------- grouped weight prep ----------------
            def weight_prep(gi, first=False):
                s0, s1 = GROUPS[gi]
                g = s1 - s0
                t_e0 = w0g.tile([P1, GMAX * W0C], io_dt, tag="t_e0")
                nc.sync.dma_start(t_e0[:, : g * W0C],
                                  we0AT[s0 * W0C: s1 * W0C, :],
                                  transpose=True)
                if first:
                    nc.sync.dma_start(t_wm0[:], wm0[:, :])
                t_e1 = w1g.tile([P1, GMAX * W1C], io_dt, tag="t_e1")
                nc.sync.dma_start(t_e1[:, : g * W1C],
                                  we1AT[s0 * W1C: s1 * W1C, :],
                                  transpose=True)
                if first:
                    nc.sync.dma_start(t_wm1[:], wm1[:, :])

                t_wl = wls.tile([P1, GMAX * T1 * DO], io_dt, tag="t_wlf")
                for j in range(g):
                    s = s0 + j
                    w0j = t_e0[: P0, j * W0C: (j + 1) * W0C]
                    nc.vector.tensor_mul(w0j, w0j, t_std0[:])
                    w1j = t_e1[:, j * W1C: (j + 1) * W1C]
                    nc.vector.tensor_mul(w1j, w1j, t_std1[:])
                    nc.vector.tensor_add(w1j[:, :H1], w1j[:, :H1],
                                         t_wm1[:, :H1])
                    nc.gpsimd.tensor_add(w1j[:, H1:], w1j[:, H1:],
                                         t_wm1[:, H1:])
                    wlj = t_wl[:, j * T1 * DO: (j + 1) * T1 * DO]
                    nc.vector.tensor_mul(
                        wlj, t_wel[:, s * T1 * DO: (s + 1) * T1 * DO],
                        t_stdl[:])
                    nc.vector.tensor_add(wlj, wlj, t_wml[:])
                return t_e0, t_e1, t_wl

            def compute(s, gi, t_e0, t_e1, t_wl, t_y0T, po):
                s0 = GROUPS[gi][0]
                j = s - s0
                w0 = t_e0[: P0, j * W0C: (j + 1) * W0C]
                w1 = t_e1[:, j * W1C: (j + 1) * W1C]
                wlf = t_wl[:, j * T1 * DO: (j + 1) * T1 * DO]

                # layer 0: psum initialized with y0T via identity matmul
                p0 = ps0.tile([P1, C1 * B], F32, tag="p0")
                a1T = acts.tile([P1, C1 * B], io_dt, tag="a1T")
                for c in range(C1):
                    mm(p0[:, ts(c, B)], t_eye[:], t_y0T[:, ts(c, B)],
                       start=True, stop=False)
                    for t in range(T0):
                        mm(p0[:, ts(c, B)],
                           w0[:, t * D1 + c * P1: t * D1 + (c + 1) * P1],
                           t_xT[:, ts(t, B)],
                           start=False, stop=(t == T0 - 1))
                    nc.scalar.activation(
                        a1T[:, ts(c, B)], p0[:, ts(c, B)], AF.Relu,
                        bias=t_bT0[:, c * SP + s: c * SP + s + 1])

                # layer 1 (mean already folded into w1)
                p1 = ps1.tile([P1, C1 * B], F32, tag="p1")
                a2T = acts.tile([P1, C1 * B], io_dt, tag="a2T")
                for c in range(C1):
                    for t in range(T1):
                        mm(p1[:, ts(c, B)],
                           w1[:, t * D2 + c * P1: t * D2 + (c + 1) * P1],
                           a1T[:, ts(t, B)],
                           start=(t == 0), stop=(t == T1 - 1))
                    nc.scalar.activation(
                        a2T[:, ts(c, B)], p1[:, ts(c, B)], AF.Relu,
                        bias=t_bT1[:, c * SP + s: c * SP + s + 1])

                # output layer: all samples share one [64, SP*DO] psum bank
                for t in range(T1):
                    mm(po[:, ts(s, DO)], a2T[:, ts(t, B)],
                       wlf[:, ts(t, DO)], start=(t == 0), stop=False)
                mm(po[:, ts(s, DO)], t_ind[:, ts(s, B)], t_bl[:],
                   start=False, stop=True)

            po = ps_o.tile([B, SP * DO], F32, tag="out")
            NG = len(GROUPS)
            preps = [weight_prep(0, first=True)]
            t_y0T = make_y0T()
            preps.append(weight_prep(1))
            for gi in range(NG):
                for s in range(*GROUPS[gi]):
                    compute(s, gi, *preps[gi], t_y0T, po)
                if gi + 2 < NG:
                    preps.append(weight_prep(gi + 2))

            nc.scalar.copy(t_out[:], po[:])
            nc.sync.dma_start(out[:, :], t_out[:])

    nc.compile()
    return nc


def _get_nc(mode="bf16"):
    if "nc" not in _CACHE:
        _CACHE["nc"] = _build()
    return _CACHE["nc"]


def _prep_in_maps(inputs, mode="bf16"):
    import ml_dtypes
    np_dt = ml_dtypes.bfloat16

    def cvt(a):
        return np.ascontiguousarray(a).astype(np_dt, copy=False)

    x = np.asarray(inputs["inputs"], np.float32)
    we0 = np.asarray(inputs["we0"], np.float32)
    we1 = np.asarray(inputs["we1"], np.float32)
    wel = np.asarray(inputs["wel"], np.float32)
    be0 = np.asarray(inputs["be0"], np.float32).reshape(S, D1)
    be1 = np.asarray(inputs["be1"], np.float32).reshape(S, D2)
    bel = np.asarray(inputs["bel"], np.float32).reshape(S, DO)

    # p-major rows + mod-4 interleaved feature columns:
    #   out[p, (t, c, q)] = M[T*p + t, 4*q + c]
    def pm0(M):  # [784, 512] -> [112, 7*512]
        return M.reshape(P0, T0, P1, C1).transpose(0, 1, 3, 2) \
                .reshape(P0, W0C)

    def pm1(M):  # [512, 512] -> [128, 4*512]
        return M.reshape(P1, T1, P1, C1).transpose(0, 1, 3, 2) \
                .reshape(P1, W1C)

    def pml(M):  # [512, 10] -> [128, 4*10] (row permutation only)
        return M.reshape(P1, T1 * DO)

    xTpm = x.T.reshape(P0, T0, B).reshape(P0, T0 * B)

    def bias_T(b):  # [SP, D] -> [128, C1*SP] with [q, c*SP+s] = b[s, 4q+c]
        return np.ascontiguousarray(
            b.reshape(SP, P1, C1).transpose(1, 2, 0).reshape(P1, C1 * SP))

    def bias_cq(v):  # [D] -> [128, C1] with [q, c] = v[4q+c]
        return np.ascontiguousarray(np.asarray(v, np.float32)
                                    .reshape(P1, C1))

    shared = {
        "xT": cvt(xTpm),
        "wm0": cvt(pm0(np.asarray(inputs["wm0"], np.float32))),
        "wv0": cvt(pm0(np.asarray(inputs["wv0"], np.float32))),
        "wm1": cvt(pm1(np.asarray(inputs["wm1"], np.float32))),
        "wv1": cvt(pm1(np.asarray(inputs["wv1"], np.float32))),
        "wmlT": cvt(pml(np.asarray(inputs["wml"], np.float32))),
        "wvlT": cvt(pml(np.asarray(inputs["wvl"], np.float32))),
        "eye128": cvt(np.eye(P1, dtype=np.float32)),
        "bvl": cvt(np.asarray(inputs["bvl"], np.float32).reshape(1, DO)),
        "bml": cvt(np.asarray(inputs["bml"], np.float32).reshape(1, DO)),
        "ones13": cvt(np.ones((1, SP), np.float32)),
        "ind": cvt(np.repeat(np.eye(SP, dtype=np.float32), B, axis=1)),
    }

    def shard(a, k):
        lo = k * SP
        hi = lo + SP
        if hi <= S:
            return a[lo:hi]
        return np.concatenate([a[lo:S], a[: hi - S]], axis=0)

    def packT0(ws):  # list of pm0 [112, W0C] -> [SP*W0C, 128] (pad cols)
        A = np.stack(ws, axis=1).reshape(P0, SP * W0C)  # [112, SP*W0C]
        out = np.zeros((SP * W0C, P1), np.float32)
        out[:, :P0] = A.T
        return cvt(out)

    def packT1(ws):  # list of pm1 [128, W1C] -> [SP*W1C, 128]
        A = np.stack(ws, axis=1).reshape(P1, SP * W1C)
        return cvt(np.ascontiguousarray(A.T))

    in_maps = []
    for k in range(NCORES):
        welk = shard(wel, k)  # [SP, 512, 10]
        b0 = np.concatenate([bias_cq(inputs["bv0"]), bias_cq(inputs["bm0"]),
                             bias_T(shard(be0, k))], axis=1)
        b1 = np.concatenate([bias_cq(inputs["bv1"]), bias_cq(inputs["bm1"]),
                             bias_T(shard(be1, k))], axis=1)
        in_maps.append(dict(
            shared,
            we0AT=packT0([pm0(m) for m in shard(we0, k)]),
            we1AT=packT1([pm1(m) for m in shard(we1, k)]),
            welT=cvt(np.stack([pml(m) for m in welk], axis=1)
                     .reshape(P1, SP * T1 * DO)),
            b01=np.ascontiguousarray(np.concatenate([b0, b1], axis=1)),
            bel=cvt(shard(bel, k)),
        ))
    return in_maps


def _run(inputs, mode="bf16", trace=False):
    nc = _get_nc(mode)
    in_maps = _prep_in_maps(inputs, mode)
    res = run_bass_kernel_spmd(nc, in_maps, core_ids=list(range(NCORES)),
                               trace=trace)
    outs = []
    for k in range(NCORES):
        o = np.asarray(res.results[k]["out"], np.float32)  # [64, 130]
        outs.append(o.reshape(B, SP, DO).transpose(1, 0, 2))
    full = np.concatenate(outs, axis=0)[:S]  # [100, 64, 10]
    return full, res


def kernel(**inputs):
    out, _ = _run(inputs)
    return out
